# revision 14
# baseline (speedup 1.0000x reference)
"""MoE kernel v6: v3 pair structure + fp8 DoubleRow for low-weight jobs.

Structure (per pair of cores, F split in half as v3): each core holds the
F-half of BOTH its pair's experts. Jobs are split by router combine
weight: second-choice jobs with cw < THETA contribute little to the
output, so they run in fp8 e4m3 with DoubleRow perf mode (2 k-chunks per
matmul pass, ~1.5x faster); everything else stays bf16. Weights for the
fp8 path are pre-scaled by 128 on host (w values ~1/32 std would hit
e4m3 subnormals); the scale is undone by the activation's input scale
(mm1) and on host (mm2 output).

Phases per core: A (big expert, bf16) -> C (big cheap, fp8) -> D (small
cheap, fp8) -> B (small expert, bf16). B's bf16 weights load into the
SBUF region released by A's weights while the fp8 phases compute, so
both bf16 weight sets plus the fp8 set never need to be resident
together. A few warm-up matmuls on zeroed scratch SBUF run at the head
of the PE stream to cover the initial DMA window and warm the HAM clock
gate; DMAs are emitted smallest-needed-first.

Slot capacities (CA..CD) are maxima across pairs so the program is SPMD;
per-core data decides which experts a core serves.

DRAM layouts per core (FL = F/2 = 2048, FLO = 16, KO = 8):
  xa  [nA+nB, 128, KO, CT] bf16   A tiles then B tiles; xa[t,p,ko,j] = xf[tok, ko*128+p]
  xc  [nC+nD, 128, 4, 2, CT] f8e4 C tiles then D; xc[t,p,j,i,c] = xf[tok, (2j+i)*128+p]
  w1a/w1b [128, 4, KO, 512] bf16  w1_e[ko*128+p, h*FL+q*512+ff]
  w2a/w2b [2, 128, 8, D] bf16     w2_e[h*FL+(b*8+fi)*128+p, d]
  w1c [2, 128, FLO, 4, 2, 128] f8 128*w1_{e_s}[(2j+i)*128+p, h*FL+fo*128+m]
  w2c [2, 128, 8, 2, D] f8        128*w2_{e_s}[h*FL+(2fj+i)*128+p, d]
  b1a/b1b [128, FLO] f32; b1c [128, 2, FLO] f32 (true bias)
  y   [nA+nC+nD+nB, 128, KO, CT] bf16  partials; fp8-phase tiles carry 128x scale
"""

import numpy as np
import ml_dtypes

N_CORES = 8
D = 1024
F = 4096
E = 8
KO = D // 128
FL = F // 2
FLO = FL // 128      # 16
CT = 512
THETA = 0.38
SCALE = 128.0
N_WARM = 12

BF16 = ml_dtypes.bfloat16
F8 = ml_dtypes.float8_e4m3

_NC_CACHE: dict[tuple, object] = {}
LAST_RESULTS = None


def _cap_tiles(C):
    # Equal-width tiles (even widths): narrow remainder tiles are
    # LDWEIGHTS-bound on the PE, so spread the columns evenly instead.
    n = -(-C // CT)
    base = (C // n) & ~1
    rem = C - base * n
    widths = [base + 2 if i < rem // 2 else base for i in range(n)]
    tiles = []
    off = 0
    for w in widths:
        tiles.append((off, w))
        off += w
    assert off == C
    return tiles


def _build(caps):
    import concourse.mybir as mybir
    from concourse import bacc
    from concourse.tile import TileContext

    fp32 = mybir.dt.float32
    bf16 = mybir.dt.bfloat16
    f8e4 = mybir.dt.float8e4
    DR = mybir.MatmulPerfMode.DoubleRow

    CA, CB, CC, CD = caps
    tA, tB, tC, tD = (_cap_tiles(c) for c in caps)
    nA, nB, nC, nD = len(tA), len(tB), len(tC), len(tD)
    n_tot = nA + nB + nC + nD

    nc = bacc.Bacc(
        "TRN2", target_bir_lowering=False, debug=False, num_devices=N_CORES
    )
    xa = nc.dram_tensor("xa", [nA + nB, 128, KO, CT], bf16, kind="ExternalInput")
    xc = nc.dram_tensor("xc", [nC + nD, 128, 4, 2, CT], f8e4, kind="ExternalInput")
    w1a = nc.dram_tensor("w1a", [128, 4, KO, 512], bf16, kind="ExternalInput")
    w1b = nc.dram_tensor("w1b", [128, 4, KO, 512], bf16, kind="ExternalInput")
    w2a = nc.dram_tensor("w2a", [2, 128, 8, D], bf16, kind="ExternalInput")
    w2b = nc.dram_tensor("w2b", [2, 128, 8, D], bf16, kind="ExternalInput")
    w1c = nc.dram_tensor("w1c", [2, 128, FLO, 4, 2, 128], f8e4, kind="ExternalInput")
    w2c = nc.dram_tensor("w2c", [2, 128, 8, 2, D], f8e4, kind="ExternalInput")
    b1a = nc.dram_tensor("b1a", [128, FLO], fp32, kind="ExternalInput")
    b1b = nc.dram_tensor("b1b", [128, FLO], fp32, kind="ExternalInput")
    b1c = nc.dram_tensor("b1c", [128, 2, FLO], fp32, kind="ExternalInput")
    y = nc.dram_tensor("y", [n_tot, 128, KO, CT], bf16, kind="ExternalOutput")

    with TileContext(nc) as tc:
        with (
            tc.tile_pool(name="wg", bufs=1) as wg,
            tc.tile_pool(name="wcd", bufs=1) as wcd,
            tc.tile_pool(name="xpool", bufs=4) as xpool,
            tc.tile_pool(name="xcpool", bufs=2) as xcpool,
            tc.tile_pool(name="hpool", bufs=1) as hpool,
            tc.tile_pool(name="hcpool", bufs=1) as hcpool,
            tc.tile_pool(name="ypool", bufs=4) as ypool,
            tc.tile_pool(name="ph", bufs=3, space="PSUM") as phpool,
            tc.tile_pool(name="py", bufs=3, space="PSUM") as pypool,
            tc.tile_pool(name="pw", bufs=1, space="PSUM") as pwpool,
        ):
            b1a_sb = wg.tile([128, FLO], fp32)
            b1b_sb = wg.tile([128, FLO], fp32)
            b1c_sb = wg.tile([128, 2, FLO], fp32)
            warm_x = wg.tile([128, CT], bf16)
            w1c_sb = wcd.tile([128, 2, FLO, 4, 2, 128], f8e4)
            w2c_sb = wcd.tile([128, 2, 8, 2, D], f8e4)

            nc.vector.memset(warm_x[:], 0.0)
            warm_ps = pwpool.tile([128, CT], fp32)
            for _ in range(N_WARM):
                nc.tensor.matmul(
                    warm_ps[:],
                    lhsT=warm_x[:, 0:128],
                    rhs=warm_x[:],
                    start=True,
                    stop=True,
                )

            def bf16_tile(x_sb, tw, w1_sb, w2_sb, b1_sb, yti):
                h_sb = hpool.tile([128, FLO, CT], bf16)
                for fo in range(FLO):
                    q, fq = divmod(fo, 4)
                    ph = phpool.tile([128, CT], fp32)
                    for ko in range(KO):
                        nc.tensor.matmul(
                            ph[:, :tw],
                            lhsT=w1_sb[:, q, ko, fq * 128 : (fq + 1) * 128],
                            rhs=x_sb[:, ko, :tw],
                            start=(ko == 0),
                            stop=(ko == KO - 1),
                        )
                    nc.scalar.activation(
                        h_sb[:, fo, :tw],
                        ph[:, :tw],
                        mybir.ActivationFunctionType.Gelu,
                        bias=b1_sb[:, fo : fo + 1],
                    )
                for do in range(KO):
                    py = pypool.tile([128, CT], fp32)
                    for fi in range(FLO):
                        nc.tensor.matmul(
                            py[:, :tw],
                            lhsT=w2_sb[:, fi, do * 128 : (do + 1) * 128],
                            rhs=h_sb[:, fi, :tw],
                            start=(fi == 0),
                            stop=(fi == FLO - 1),
                        )
                    y_do = ypool.tile([128, CT], bf16, tag="y_do")
                    nc.vector.tensor_copy(y_do[:, :tw], py[:, :tw])
                    nc.sync.dma_start(y[yti][:, do, :], y_do[:])

            def fp8_tile(xc_sb, tw, s, yti):
                hc = hcpool.tile([128, 8, 2, CT], f8e4)
                for fo in range(FLO):
                    ph = phpool.tile([128, CT], fp32)
                    for j in range(4):
                        nc.tensor.matmul(
                            ph[:, :tw],
                            lhsT=w1c_sb[:, s, fo, j],
                            rhs=xc_sb[:, j, :, :tw],
                            start=(j == 0),
                            stop=(j == 3),
                            perf_mode=DR,
                        )
                    nc.scalar.activation(
                        hc[:, fo // 2, fo % 2, :tw],
                        ph[:, :tw],
                        mybir.ActivationFunctionType.Gelu,
                        bias=b1c_sb[:, s, fo : fo + 1],
                        scale=1.0 / SCALE,
                    )
                for do in range(KO):
                    py = pypool.tile([128, CT], fp32)
                    for fj in range(8):
                        nc.tensor.matmul(
                            py[:, :tw],
                            lhsT=w2c_sb[:, s, fj, :, do * 128 : (do + 1) * 128],
                            rhs=hc[:, fj, :, :tw],
                            start=(fj == 0),
                            stop=(fj == 7),
                            perf_mode=DR,
                        )
                    y_do = ypool.tile([128, CT], bf16, tag="y_do")
                    nc.vector.tensor_copy(y_do[:, :tw], py[:, :tw])
                    nc.sync.dma_start(y[yti][:, do, :], y_do[:])

            # ---- Phase A (big expert, bf16) ----
            with tc.tile_pool(name="wa", bufs=1) as wa:
                w1a_sb = wa.tile([128, 4, KO, 512], bf16)
                w2a_sb = wa.tile([128, 16, D], bf16)

                # Startup-critical DMAs, smallest-needed-first: the first
                # matmul chain group (q=0) consumes x ko-chunks in order
                # with the matching 128-col weight chunks.
                x_first = xpool.tile([128, KO, CT], bf16, tag="x_sb")
                nc.sync.dma_start(x_first[:, 0:2, :], xa[0][:, 0:2, :])
                nc.sync.dma_start(w1a_sb[:, 0, 0:2], w1a[:, 0, 0:2])
                nc.sync.dma_start(x_first[:, 2:4, :], xa[0][:, 2:4, :])
                nc.sync.dma_start(w1a_sb[:, 0, 2:4], w1a[:, 0, 2:4])
                nc.sync.dma_start(x_first[:, 4:8, :], xa[0][:, 4:8, :])
                nc.sync.dma_start(w1a_sb[:, 0, 4:8], w1a[:, 0, 4:8])
                nc.sync.dma_start(b1a_sb[:], b1a[:])
                nc.sync.dma_start(b1b_sb[:], b1b[:])
                nc.sync.dma_start(b1c_sb[:], b1c[:])
                for q2 in range(1, 4):
                    nc.sync.dma_start(w1a_sb[:, q2], w1a[:, q2])
                xa_pre = [x_first]
                for ti in range(1, min(4, nA)):
                    t = xpool.tile([128, KO, CT], bf16, tag="x_sb")
                    nc.sync.dma_start(t[:], xa[ti])
                    xa_pre.append(t)
                    if ti == 1:
                        for b in range(2):
                            nc.sync.dma_start(
                                w2a_sb[:, b * 8 : (b + 1) * 8, :], w2a[b]
                            )
                xc_pre = []
                for ti in range(min(2, nC + nD)):
                    t = xcpool.tile([128, 4, 2, CT], f8e4, tag="xc_sb")
                    nc.sync.dma_start(t[:], xc[ti])
                    xc_pre.append(t)
                for s in range(2):
                    nc.sync.dma_start(w1c_sb[:, s], w1c[s])
                    nc.sync.dma_start(w2c_sb[:, s], w2c[s])

                for ti, (off, tw) in enumerate(tA):
                    if ti < len(xa_pre):
                        x_sb = xa_pre[ti]
                    else:
                        x_sb = xpool.tile([128, KO, CT], bf16, tag="x_sb")
                        nc.sync.dma_start(x_sb[:], xa[ti])
                    bf16_tile(x_sb, tw, w1a_sb, w2a_sb, b1a_sb, ti)

            # ---- Phases C, D (fp8) with B weights loading into A's space ----
            with tc.tile_pool(name="wb", bufs=1) as wb:
                w1b_sb = wb.tile([128, 4, KO, 512], bf16)
                w2b_sb = wb.tile([128, 16, D], bf16)
                for q2 in range(4):
                    nc.sync.dma_start(w1b_sb[:, q2], w1b[:, q2])
                for b in range(2):
                    nc.sync.dma_start(
                        w2b_sb[:, b * 8 : (b + 1) * 8, :], w2b[b]
                    )
                xb_pre = []
                for ti in range(min(2, nB)):
                    t = xpool.tile([128, KO, CT], bf16, tag="x_sb")
                    nc.sync.dma_start(t[:], xa[nA + ti])
                    xb_pre.append(t)

                for ci, (off, tw) in enumerate(tC + tD):
                    s = 0 if ci < nC else 1
                    if ci < len(xc_pre):
                        xc_sb = xc_pre[ci]
                    else:
                        xc_sb = xcpool.tile([128, 4, 2, CT], f8e4, tag="xc_sb")
                        nc.sync.dma_start(xc_sb[:], xc[ci])
                    fp8_tile(xc_sb, tw, s, nA + nB + ci)

                # ---- Phase B (small expert, bf16) ----
                for ti, (off, tw) in enumerate(tB):
                    if ti < len(xb_pre):
                        x_sb = xb_pre[ti]
                    else:
                        x_sb = xpool.tile([128, KO, CT], bf16, tag="x_sb")
                        nc.sync.dma_start(x_sb[:], xa[nA + ti])
                    bf16_tile(x_sb, tw, w1b_sb, w2b_sb, b1b_sb, nA + ti)

    nc.compile()
    return nc


def kernel(x, gate_w, w1, b1, w2, b2):
    from concourse.bass_utils import run_bass_kernel_spmd
    import itertools

    global LAST_RESULTS

    x = np.asarray(x, dtype=np.float32)
    gate_w = np.asarray(gate_w, dtype=np.float32)
    w1 = np.asarray(w1, dtype=np.float32)
    b1 = np.asarray(b1, dtype=np.float32)
    w2 = np.asarray(w2, dtype=np.float32)
    b2 = np.asarray(b2, dtype=np.float32)

    B, S, Din = x.shape
    assert Din == D and gate_w.shape == (D, E)
    T = B * S
    xf = x.reshape(T, D)

    # ---- Host router ----
    logits = xf.astype(np.float64) @ gate_w.astype(np.float64)
    idx0 = np.argmax(logits, axis=1)
    rows = np.arange(T)
    v0 = logits[rows, idx0]
    l2 = logits.copy()
    l2[rows, idx0] = -np.inf
    idx1 = np.argmax(l2, axis=1)
    v1_ = l2[rows, idx1]
    e1 = np.exp(v1_ - v0)
    cw0 = 1.0 / (1.0 + e1)
    cw1 = e1 / (1.0 + e1)

    exp_ids, exp_w, ch_ids, ch_w = [], [], [], []
    for e in range(E):
        sel0 = idx0 == e
        sel1e = (idx1 == e) & (cw1 >= THETA)
        sel1c = (idx1 == e) & (cw1 < THETA)
        ids = np.nonzero(sel0 | sel1e)[0]
        exp_ids.append(ids)
        exp_w.append(np.where(sel0[ids], cw0[ids], cw1[ids]))
        ids = np.nonzero(sel1c)[0]
        ch_ids.append(ids)
        ch_w.append(cw1[ids])
    ex = np.array([len(i) for i in exp_ids])
    ch = np.array([len(i) for i in ch_ids])

    # ---- Pairing: minimize 256*(CA+CB) + 178*(CC+CD) ----
    best = None
    for perm in itertools.permutations(range(E)):
        pairs = [(perm[0], perm[1]), (perm[2], perm[3]),
                 (perm[4], perm[5]), (perm[6], perm[7])]
        CA = max(ex[a] for a, _ in pairs)
        CB = max(ex[b] for _, b in pairs)
        CC = max(ch[a] for a, _ in pairs)
        CD = max(ch[b] for _, b in pairs)
        cost = 256 * (CA + CB) + 178 * (CC + CD)
        if best is None or cost < best[0]:
            best = (cost, pairs)
    pairs = best[1]
    CA = int(max(ex[a] for a, _ in pairs)); CA += CA & 1
    CB = int(max(ex[b] for _, b in pairs)); CB += CB & 1
    CC = int(max(ch[a] for a, _ in pairs)); CC += CC & 1
    CD = int(max(ch[b] for _, b in pairs)); CD += CD & 1
    caps = (CA, CB, CC, CD)

    if caps not in _NC_CACHE:
        _NC_CACHE[caps] = _build(caps)
    nc = _NC_CACHE[caps]

    tA, tB, tC, tD = (_cap_tiles(c) for c in caps)
    nA, nB, nC, nD = len(tA), len(tB), len(tC), len(tD)
    n_tot = nA + nB + nC + nD

    def pack_bf16(ids_seg):
        n = len(ids_seg)
        return xf[ids_seg].astype(BF16).reshape(n, KO, 128).transpose(2, 1, 0)

    def pack_f8(ids_seg):
        n = len(ids_seg)
        return (
            xf[ids_seg].astype(F8).reshape(n, 4, 2, 128).transpose(3, 1, 2, 0)
        )

    in_maps = [None] * N_CORES
    for pi, (eA, eB) in enumerate(pairs):
        xa_t = np.zeros((nA + nB, 128, KO, CT), dtype=BF16)
        for ti, (off, tw) in enumerate(tA):
            seg = exp_ids[eA][off : off + tw]
            if len(seg):
                xa_t[ti, :, :, : len(seg)] = pack_bf16(seg)
        for ti, (off, tw) in enumerate(tB):
            seg = exp_ids[eB][off : off + tw]
            if len(seg):
                xa_t[nA + ti, :, :, : len(seg)] = pack_bf16(seg)
        xc_t = np.zeros((nC + nD, 128, 4, 2, CT), dtype=F8)
        for ti, (off, tw) in enumerate(tC):
            seg = ch_ids[eA][off : off + tw]
            if len(seg):
                xc_t[ti, :, :, :, : len(seg)] = pack_f8(seg)
        for ti, (off, tw) in enumerate(tD):
            seg = ch_ids[eB][off : off + tw]
            if len(seg):
                xc_t[nC + ti, :, :, :, : len(seg)] = pack_f8(seg)
        xa_t = np.ascontiguousarray(xa_t)
        xc_t = np.ascontiguousarray(xc_t)

        for h in range(2):
            sl = slice(h * FL, (h + 1) * FL)
            w1a_c = np.ascontiguousarray(
                w1[eA][:, sl].reshape(KO, 128, 4, 512).transpose(1, 2, 0, 3).astype(BF16)
            )
            w1b_c = np.ascontiguousarray(
                w1[eB][:, sl].reshape(KO, 128, 4, 512).transpose(1, 2, 0, 3).astype(BF16)
            )
            w2a_c = np.ascontiguousarray(
                w2[eA][sl, :].reshape(2, 8, 128, D).transpose(0, 2, 1, 3).astype(BF16)
            )
            w2b_c = np.ascontiguousarray(
                w2[eB][sl, :].reshape(2, 8, 128, D).transpose(0, 2, 1, 3).astype(BF16)
            )
            w1c_c = np.ascontiguousarray(
                np.stack(
                    [
                        (SCALE * w1[e][:, sl])
                        .astype(F8)
                        .reshape(4, 2, 128, FLO, 128)
                        .transpose(2, 3, 0, 1, 4)
                        for e in (eA, eB)
                    ]
                )
            )  # [2, 128, FLO, 4, 2, 128]
            w2c_c = np.ascontiguousarray(
                np.stack(
                    [
                        (SCALE * w2[e][sl, :])
                        .astype(F8)
                        .reshape(8, 2, 128, D)
                        .transpose(2, 0, 1, 3)
                        for e in (eA, eB)
                    ]
                )
            )  # [2, 128, 8, 2, D]
            b1a_c = np.ascontiguousarray(b1[eA][sl].reshape(FLO, 128).T)
            b1b_c = np.ascontiguousarray(b1[eB][sl].reshape(FLO, 128).T)
            b1c_c = np.ascontiguousarray(
                np.stack([b1[e][sl].reshape(FLO, 128).T for e in (eA, eB)], axis=1)
            )  # [128, 2, FLO]
            in_maps[2 * pi + h] = {
                "xa": xa_t,
                "xc": xc_t,
                "w1a": w1a_c,
                "w1b": w1b_c,
                "w2a": w2a_c,
                "w2b": w2b_c,
                "w1c": w1c_c,
                "w2c": w2c_c,
                "b1a": b1a_c,
                "b1b": b1b_c,
                "b1c": b1c_c,
            }

    res = run_bass_kernel_spmd(nc, in_maps, core_ids=list(range(N_CORES)))
    LAST_RESULTS = res

    # ---- Host combine ----
    out = np.zeros((T, D), dtype=np.float32)
    for pi, (eA, eB) in enumerate(pairs):
        ysum = res.results[2 * pi]["y"].astype(np.float32) + res.results[
            2 * pi + 1
        ]["y"].astype(np.float32)

        def scatter(ti, ids_seg, w_seg, e, scale):
            n = len(ids_seg)
            if n == 0:
                return
            yt = ysum[ti, :, :, :n].transpose(2, 1, 0).reshape(n, D)
            out[ids_seg] += w_seg[:, None].astype(np.float32) * (
                yt * scale + b2[e]
            )

        for ti, (off, tw) in enumerate(tA):
            scatter(ti, exp_ids[eA][off : off + tw], exp_w[eA][off : off + tw], eA, 1.0)
        for ti, (off, tw) in enumerate(tC):
            scatter(nA + nB + ti, ch_ids[eA][off : off + tw], ch_w[eA][off : off + tw], eA, 1.0 / SCALE)
        for ti, (off, tw) in enumerate(tD):
            scatter(nA + nB + nC + ti, ch_ids[eB][off : off + tw], ch_w[eB][off : off + tw], eB, 1.0 / SCALE)
        for ti, (off, tw) in enumerate(tB):
            scatter(nA + ti, exp_ids[eB][off : off + tw], exp_w[eB][off : off + tw], eB, 1.0)

    return out.reshape(B, S, D)


# revision 15
# speedup vs baseline: 1.0508x; 1.0508x over previous
"""MoE kernel v6: v3 pair structure + fp8 DoubleRow for low-weight jobs.

Structure (per pair of cores, F split in half as v3): each core holds the
F-half of BOTH its pair's experts. Jobs are split by router combine
weight: second-choice jobs with cw < THETA contribute little to the
output, so they run in fp8 e4m3 with DoubleRow perf mode (2 k-chunks per
matmul pass, ~1.5x faster); everything else stays bf16. Weights for the
fp8 path are pre-scaled by 128 on host (w values ~1/32 std would hit
e4m3 subnormals); the scale is undone by the activation's input scale
(mm1) and on host (mm2 output).

Phases per core: A (big expert, bf16) -> C (big cheap, fp8) -> D (small
cheap, fp8) -> B (small expert, bf16). B's bf16 weights load into the
SBUF region released by A's weights while the fp8 phases compute, so
both bf16 weight sets plus the fp8 set never need to be resident
together. A few warm-up matmuls on zeroed scratch SBUF run at the head
of the PE stream to cover the initial DMA window and warm the HAM clock
gate; DMAs are emitted smallest-needed-first.

Slot capacities (CA..CD) are maxima across pairs so the program is SPMD;
per-core data decides which experts a core serves.

DRAM layouts per core (FL = F/2 = 2048, FLO = 16, KO = 8):
  xa  [nA+nB, 128, KO, CT] bf16   A tiles then B tiles; xa[t,p,ko,j] = xf[tok, ko*128+p]
  xc  [nC+nD, 128, 4, 2, CT] f8e4 C tiles then D; xc[t,p,j,i,c] = xf[tok, (2j+i)*128+p]
  w1a/w1b [128, 4, KO, 512] bf16  w1_e[ko*128+p, h*FL+q*512+ff]
  w2a/w2b [2, 128, 8, D] bf16     w2_e[h*FL+(b*8+fi)*128+p, d]
  w1c [2, 128, FLO, 4, 2, 128] f8 128*w1_{e_s}[(2j+i)*128+p, h*FL+fo*128+m]
  w2c [2, 128, 8, 2, D] f8        128*w2_{e_s}[h*FL+(2fj+i)*128+p, d]
  b1a/b1b [128, FLO] f32; b1c [128, 2, FLO] f32 (true bias)
  y   [nA+nC+nD+nB, 128, KO, CT] bf16  partials; fp8-phase tiles carry 128x scale
"""

import numpy as np
import ml_dtypes

N_CORES = 8
D = 1024
F = 4096
E = 8
KO = D // 128
FL = F // 2
FLO = FL // 128      # 16
CT = 512
THETA = 0.38
SCALE = 128.0
N_WARM = 12

BF16 = ml_dtypes.bfloat16
F8 = ml_dtypes.float8_e4m3

_NC_CACHE: dict[tuple, object] = {}
LAST_RESULTS = None


def _cap_tiles(C):
    # Equal-width tiles (even widths): narrow remainder tiles are
    # LDWEIGHTS-bound on the PE, so spread the columns evenly instead.
    n = -(-C // CT)
    base = (C // n) & ~1
    rem = C - base * n
    widths = [base + 2 if i < rem // 2 else base for i in range(n)]
    tiles = []
    off = 0
    for w in widths:
        tiles.append((off, w))
        off += w
    assert off == C
    return tiles


def _build(caps):
    import concourse.mybir as mybir
    from concourse import bacc
    from concourse.tile import TileContext

    fp32 = mybir.dt.float32
    bf16 = mybir.dt.bfloat16
    f8e4 = mybir.dt.float8e4
    DR = mybir.MatmulPerfMode.DoubleRow

    CA, CB, CC, CD = caps
    tA, tB, tC, tD = (_cap_tiles(c) for c in caps)
    nA, nB, nC, nD = len(tA), len(tB), len(tC), len(tD)
    n_tot = nA + nB + nC + nD

    nc = bacc.Bacc(
        "TRN2", target_bir_lowering=False, debug=False, num_devices=N_CORES
    )
    xa = nc.dram_tensor("xa", [nA + nB, 128, KO, CT], bf16, kind="ExternalInput")
    xc = nc.dram_tensor("xc", [nC + nD, 128, 4, 2, CT], f8e4, kind="ExternalInput")
    w1a = nc.dram_tensor("w1a", [128, 4, KO, 512], bf16, kind="ExternalInput")
    w1b = nc.dram_tensor("w1b", [128, 4, KO, 512], bf16, kind="ExternalInput")
    w2a = nc.dram_tensor("w2a", [2, 128, 8, D], bf16, kind="ExternalInput")
    w2b = nc.dram_tensor("w2b", [2, 128, 8, D], bf16, kind="ExternalInput")
    w1c = nc.dram_tensor("w1c", [2, 128, FLO, 4, 2, 128], f8e4, kind="ExternalInput")
    w2c = nc.dram_tensor("w2c", [2, 128, 8, 2, D], f8e4, kind="ExternalInput")
    b1a = nc.dram_tensor("b1a", [128, FLO], fp32, kind="ExternalInput")
    b1b = nc.dram_tensor("b1b", [128, FLO], fp32, kind="ExternalInput")
    b1c = nc.dram_tensor("b1c", [128, 2, FLO], fp32, kind="ExternalInput")
    y = nc.dram_tensor("y", [n_tot, 128, KO, CT], bf16, kind="ExternalOutput")

    with TileContext(nc) as tc:
        with (
            tc.tile_pool(name="wg", bufs=1) as wg,
            tc.tile_pool(name="wcd", bufs=1) as wcd,
            tc.tile_pool(name="xpool", bufs=4) as xpool,
            tc.tile_pool(name="xcpool", bufs=2) as xcpool,
            tc.tile_pool(name="hpool", bufs=1) as hpool,
            tc.tile_pool(name="hcpool", bufs=1) as hcpool,
            tc.tile_pool(name="ypool", bufs=6) as ypool,
            tc.tile_pool(name="ph", bufs=3, space="PSUM") as phpool,
            tc.tile_pool(name="py", bufs=3, space="PSUM") as pypool,
            tc.tile_pool(name="pw", bufs=1, space="PSUM") as pwpool,
        ):
            b1a_sb = wg.tile([128, FLO], fp32)
            b1b_sb = wg.tile([128, FLO], fp32)
            b1c_sb = wg.tile([128, 2, FLO], fp32)
            warm_x = wg.tile([128, CT], bf16)
            w1c_sb = wcd.tile([128, 2, FLO, 4, 2, 128], f8e4)
            w2c_sb = wcd.tile([128, 2, 8, 2, D], f8e4)

            nc.vector.memset(warm_x[:], 0.0)
            warm_ps = pwpool.tile([128, CT], fp32)
            for _ in range(N_WARM):
                nc.tensor.matmul(
                    warm_ps[:],
                    lhsT=warm_x[:, 0:128],
                    rhs=warm_x[:],
                    start=True,
                    stop=True,
                )

            def bf16_tile(x_sb, tw, w1_sb, w2_sb, b1_sb, yti):
                h_sb = hpool.tile([128, FLO, CT], bf16)
                for fo in range(FLO):
                    q, fq = divmod(fo, 4)
                    ph = phpool.tile([128, CT], fp32)
                    for ko in range(KO):
                        nc.tensor.matmul(
                            ph[:, :tw],
                            lhsT=w1_sb[:, q, ko, fq * 128 : (fq + 1) * 128],
                            rhs=x_sb[:, ko, :tw],
                            start=(ko == 0),
                            stop=(ko == KO - 1),
                        )
                    nc.scalar.activation(
                        h_sb[:, fo, :tw],
                        ph[:, :tw],
                        mybir.ActivationFunctionType.Gelu,
                        bias=b1_sb[:, fo : fo + 1],
                    )
                for do in range(KO):
                    py = pypool.tile([128, CT], fp32)
                    for fi in range(FLO):
                        nc.tensor.matmul(
                            py[:, :tw],
                            lhsT=w2_sb[:, fi, do * 128 : (do + 1) * 128],
                            rhs=h_sb[:, fi, :tw],
                            start=(fi == 0),
                            stop=(fi == FLO - 1),
                        )
                    y_do = ypool.tile([128, CT], bf16, tag="y_do")
                    nc.vector.tensor_copy(y_do[:, :tw], py[:, :tw])
                    # scalar-engine DGE queue: y writebacks must not queue
                    # behind bulk x/weight loads on the sync queue
                    nc.scalar.dma_start(y[yti][:, do, :], y_do[:])

            def fp8_tile(xc_sb, tw, s, yti):
                hc = hcpool.tile([128, 8, 2, CT], f8e4)
                for fo in range(FLO):
                    ph = phpool.tile([128, CT], fp32)
                    for j in range(4):
                        nc.tensor.matmul(
                            ph[:, :tw],
                            lhsT=w1c_sb[:, s, fo, j],
                            rhs=xc_sb[:, j, :, :tw],
                            start=(j == 0),
                            stop=(j == 3),
                            perf_mode=DR,
                        )
                    nc.scalar.activation(
                        hc[:, fo // 2, fo % 2, :tw],
                        ph[:, :tw],
                        mybir.ActivationFunctionType.Gelu,
                        bias=b1c_sb[:, s, fo : fo + 1],
                        scale=1.0 / SCALE,
                    )
                for do in range(KO):
                    py = pypool.tile([128, CT], fp32)
                    for fj in range(8):
                        nc.tensor.matmul(
                            py[:, :tw],
                            lhsT=w2c_sb[:, s, fj, :, do * 128 : (do + 1) * 128],
                            rhs=hc[:, fj, :, :tw],
                            start=(fj == 0),
                            stop=(fj == 7),
                            perf_mode=DR,
                        )
                    y_do = ypool.tile([128, CT], bf16, tag="y_do")
                    nc.vector.tensor_copy(y_do[:, :tw], py[:, :tw])
                    # scalar-engine DGE queue: y writebacks must not queue
                    # behind bulk x/weight loads on the sync queue
                    nc.scalar.dma_start(y[yti][:, do, :], y_do[:])

            # ---- Phase A (big expert, bf16) ----
            with tc.tile_pool(name="wa", bufs=1) as wa:
                w1a_sb = wa.tile([128, 4, KO, 512], bf16)
                w2a_sb = wa.tile([128, 16, D], bf16)

                # Startup-critical DMAs, smallest-needed-first: the first
                # matmul chain group (q=0) consumes x ko-chunks in order
                # with the matching 128-col weight chunks.
                x_first = xpool.tile([128, KO, CT], bf16, tag="x_sb")
                nc.sync.dma_start(x_first[:, 0:2, :], xa[0][:, 0:2, :])
                nc.sync.dma_start(w1a_sb[:, 0, 0:2], w1a[:, 0, 0:2])
                nc.sync.dma_start(x_first[:, 2:4, :], xa[0][:, 2:4, :])
                nc.sync.dma_start(w1a_sb[:, 0, 2:4], w1a[:, 0, 2:4])
                nc.sync.dma_start(x_first[:, 4:8, :], xa[0][:, 4:8, :])
                nc.sync.dma_start(w1a_sb[:, 0, 4:8], w1a[:, 0, 4:8])
                nc.sync.dma_start(b1a_sb[:], b1a[:])
                nc.sync.dma_start(b1b_sb[:], b1b[:])
                nc.sync.dma_start(b1c_sb[:], b1c[:])
                for q2 in range(1, 4):
                    nc.sync.dma_start(w1a_sb[:, q2], w1a[:, q2])
                xa_pre = [x_first]
                for ti in range(1, min(4, nA)):
                    t = xpool.tile([128, KO, CT], bf16, tag="x_sb")
                    nc.sync.dma_start(t[:], xa[ti])
                    xa_pre.append(t)
                    if ti == 1:
                        for b in range(2):
                            nc.sync.dma_start(
                                w2a_sb[:, b * 8 : (b + 1) * 8, :], w2a[b]
                            )
                xc_pre = []
                for ti in range(min(2, nC + nD)):
                    t = xcpool.tile([128, 4, 2, CT], f8e4, tag="xc_sb")
                    nc.sync.dma_start(t[:], xc[ti])
                    xc_pre.append(t)
                for s in range(2):
                    nc.sync.dma_start(w1c_sb[:, s], w1c[s])
                    nc.sync.dma_start(w2c_sb[:, s], w2c[s])

                for ti, (off, tw) in enumerate(tA):
                    if ti < len(xa_pre):
                        x_sb = xa_pre[ti]
                    else:
                        x_sb = xpool.tile([128, KO, CT], bf16, tag="x_sb")
                        nc.sync.dma_start(x_sb[:], xa[ti])
                    bf16_tile(x_sb, tw, w1a_sb, w2a_sb, b1a_sb, ti)

            # ---- Phases C, D (fp8) with B weights loading into A's space ----
            with tc.tile_pool(name="wb", bufs=1) as wb:
                w1b_sb = wb.tile([128, 4, KO, 512], bf16)
                w2b_sb = wb.tile([128, 16, D], bf16)
                for q2 in range(4):
                    nc.sync.dma_start(w1b_sb[:, q2], w1b[:, q2])
                for b in range(2):
                    nc.sync.dma_start(
                        w2b_sb[:, b * 8 : (b + 1) * 8, :], w2b[b]
                    )
                xb_pre = []
                for ti in range(min(2, nB)):
                    t = xpool.tile([128, KO, CT], bf16, tag="x_sb")
                    nc.sync.dma_start(t[:], xa[nA + ti])
                    xb_pre.append(t)

                for ci, (off, tw) in enumerate(tC + tD):
                    s = 0 if ci < nC else 1
                    if ci < len(xc_pre):
                        xc_sb = xc_pre[ci]
                    else:
                        xc_sb = xcpool.tile([128, 4, 2, CT], f8e4, tag="xc_sb")
                        nc.sync.dma_start(xc_sb[:], xc[ci])
                    fp8_tile(xc_sb, tw, s, nA + nB + ci)

                # ---- Phase B (small expert, bf16) ----
                for ti, (off, tw) in enumerate(tB):
                    if ti < len(xb_pre):
                        x_sb = xb_pre[ti]
                    else:
                        x_sb = xpool.tile([128, KO, CT], bf16, tag="x_sb")
                        nc.sync.dma_start(x_sb[:], xa[nA + ti])
                    bf16_tile(x_sb, tw, w1b_sb, w2b_sb, b1b_sb, nA + ti)

    nc.compile()
    return nc


def kernel(x, gate_w, w1, b1, w2, b2):
    from concourse.bass_utils import run_bass_kernel_spmd
    import itertools

    global LAST_RESULTS

    x = np.asarray(x, dtype=np.float32)
    gate_w = np.asarray(gate_w, dtype=np.float32)
    w1 = np.asarray(w1, dtype=np.float32)
    b1 = np.asarray(b1, dtype=np.float32)
    w2 = np.asarray(w2, dtype=np.float32)
    b2 = np.asarray(b2, dtype=np.float32)

    B, S, Din = x.shape
    assert Din == D and gate_w.shape == (D, E)
    T = B * S
    xf = x.reshape(T, D)

    # ---- Host router ----
    logits = xf.astype(np.float64) @ gate_w.astype(np.float64)
    idx0 = np.argmax(logits, axis=1)
    rows = np.arange(T)
    v0 = logits[rows, idx0]
    l2 = logits.copy()
    l2[rows, idx0] = -np.inf
    idx1 = np.argmax(l2, axis=1)
    v1_ = l2[rows, idx1]
    e1 = np.exp(v1_ - v0)
    cw0 = 1.0 / (1.0 + e1)
    cw1 = e1 / (1.0 + e1)

    exp_ids, exp_w, ch_ids, ch_w = [], [], [], []
    for e in range(E):
        sel0 = idx0 == e
        sel1e = (idx1 == e) & (cw1 >= THETA)
        sel1c = (idx1 == e) & (cw1 < THETA)
        ids = np.nonzero(sel0 | sel1e)[0]
        exp_ids.append(ids)
        exp_w.append(np.where(sel0[ids], cw0[ids], cw1[ids]))
        ids = np.nonzero(sel1c)[0]
        ch_ids.append(ids)
        ch_w.append(cw1[ids])
    ex = np.array([len(i) for i in exp_ids])
    ch = np.array([len(i) for i in ch_ids])

    # ---- Pairing: minimize 256*(CA+CB) + 178*(CC+CD) ----
    best = None
    for perm in itertools.permutations(range(E)):
        pairs = [(perm[0], perm[1]), (perm[2], perm[3]),
                 (perm[4], perm[5]), (perm[6], perm[7])]
        CA = max(ex[a] for a, _ in pairs)
        CB = max(ex[b] for _, b in pairs)
        CC = max(ch[a] for a, _ in pairs)
        CD = max(ch[b] for _, b in pairs)
        cost = 256 * (CA + CB) + 178 * (CC + CD)
        if best is None or cost < best[0]:
            best = (cost, pairs)
    pairs = best[1]
    CA = int(max(ex[a] for a, _ in pairs)); CA += CA & 1
    CB = int(max(ex[b] for _, b in pairs)); CB += CB & 1
    CC = int(max(ch[a] for a, _ in pairs)); CC += CC & 1
    CD = int(max(ch[b] for _, b in pairs)); CD += CD & 1
    caps = (CA, CB, CC, CD)

    if caps not in _NC_CACHE:
        _NC_CACHE[caps] = _build(caps)
    nc = _NC_CACHE[caps]

    tA, tB, tC, tD = (_cap_tiles(c) for c in caps)
    nA, nB, nC, nD = len(tA), len(tB), len(tC), len(tD)
    n_tot = nA + nB + nC + nD

    def pack_bf16(ids_seg):
        n = len(ids_seg)
        return xf[ids_seg].astype(BF16).reshape(n, KO, 128).transpose(2, 1, 0)

    def pack_f8(ids_seg):
        n = len(ids_seg)
        return (
            xf[ids_seg].astype(F8).reshape(n, 4, 2, 128).transpose(3, 1, 2, 0)
        )

    in_maps = [None] * N_CORES
    for pi, (eA, eB) in enumerate(pairs):
        xa_t = np.zeros((nA + nB, 128, KO, CT), dtype=BF16)
        for ti, (off, tw) in enumerate(tA):
            seg = exp_ids[eA][off : off + tw]
            if len(seg):
                xa_t[ti, :, :, : len(seg)] = pack_bf16(seg)
        for ti, (off, tw) in enumerate(tB):
            seg = exp_ids[eB][off : off + tw]
            if len(seg):
                xa_t[nA + ti, :, :, : len(seg)] = pack_bf16(seg)
        xc_t = np.zeros((nC + nD, 128, 4, 2, CT), dtype=F8)
        for ti, (off, tw) in enumerate(tC):
            seg = ch_ids[eA][off : off + tw]
            if len(seg):
                xc_t[ti, :, :, :, : len(seg)] = pack_f8(seg)
        for ti, (off, tw) in enumerate(tD):
            seg = ch_ids[eB][off : off + tw]
            if len(seg):
                xc_t[nC + ti, :, :, :, : len(seg)] = pack_f8(seg)
        xa_t = np.ascontiguousarray(xa_t)
        xc_t = np.ascontiguousarray(xc_t)

        for h in range(2):
            sl = slice(h * FL, (h + 1) * FL)
            w1a_c = np.ascontiguousarray(
                w1[eA][:, sl].reshape(KO, 128, 4, 512).transpose(1, 2, 0, 3).astype(BF16)
            )
            w1b_c = np.ascontiguousarray(
                w1[eB][:, sl].reshape(KO, 128, 4, 512).transpose(1, 2, 0, 3).astype(BF16)
            )
            w2a_c = np.ascontiguousarray(
                w2[eA][sl, :].reshape(2, 8, 128, D).transpose(0, 2, 1, 3).astype(BF16)
            )
            w2b_c = np.ascontiguousarray(
                w2[eB][sl, :].reshape(2, 8, 128, D).transpose(0, 2, 1, 3).astype(BF16)
            )
            w1c_c = np.ascontiguousarray(
                np.stack(
                    [
                        (SCALE * w1[e][:, sl])
                        .astype(F8)
                        .reshape(4, 2, 128, FLO, 128)
                        .transpose(2, 3, 0, 1, 4)
                        for e in (eA, eB)
                    ]
                )
            )  # [2, 128, FLO, 4, 2, 128]
            w2c_c = np.ascontiguousarray(
                np.stack(
                    [
                        (SCALE * w2[e][sl, :])
                        .astype(F8)
                        .reshape(8, 2, 128, D)
                        .transpose(2, 0, 1, 3)
                        for e in (eA, eB)
                    ]
                )
            )  # [2, 128, 8, 2, D]
            b1a_c = np.ascontiguousarray(b1[eA][sl].reshape(FLO, 128).T)
            b1b_c = np.ascontiguousarray(b1[eB][sl].reshape(FLO, 128).T)
            b1c_c = np.ascontiguousarray(
                np.stack([b1[e][sl].reshape(FLO, 128).T for e in (eA, eB)], axis=1)
            )  # [128, 2, FLO]
            in_maps[2 * pi + h] = {
                "xa": xa_t,
                "xc": xc_t,
                "w1a": w1a_c,
                "w1b": w1b_c,
                "w2a": w2a_c,
                "w2b": w2b_c,
                "w1c": w1c_c,
                "w2c": w2c_c,
                "b1a": b1a_c,
                "b1b": b1b_c,
                "b1c": b1c_c,
            }

    res = run_bass_kernel_spmd(nc, in_maps, core_ids=list(range(N_CORES)))
    LAST_RESULTS = res

    # ---- Host combine ----
    out = np.zeros((T, D), dtype=np.float32)
    for pi, (eA, eB) in enumerate(pairs):
        ysum = res.results[2 * pi]["y"].astype(np.float32) + res.results[
            2 * pi + 1
        ]["y"].astype(np.float32)

        def scatter(ti, ids_seg, w_seg, e, scale):
            n = len(ids_seg)
            if n == 0:
                return
            yt = ysum[ti, :, :, :n].transpose(2, 1, 0).reshape(n, D)
            out[ids_seg] += w_seg[:, None].astype(np.float32) * (
                yt * scale + b2[e]
            )

        for ti, (off, tw) in enumerate(tA):
            scatter(ti, exp_ids[eA][off : off + tw], exp_w[eA][off : off + tw], eA, 1.0)
        for ti, (off, tw) in enumerate(tC):
            scatter(nA + nB + ti, ch_ids[eA][off : off + tw], ch_w[eA][off : off + tw], eA, 1.0 / SCALE)
        for ti, (off, tw) in enumerate(tD):
            scatter(nA + nB + nC + ti, ch_ids[eB][off : off + tw], ch_w[eB][off : off + tw], eB, 1.0 / SCALE)
        for ti, (off, tw) in enumerate(tB):
            scatter(nA + ti, exp_ids[eB][off : off + tw], exp_w[eB][off : off + tw], eB, 1.0)

    return out.reshape(B, S, D)
